# revision 1
# baseline (speedup 1.0000x reference)
"""Trainium2 Bass kernel for CantorAttention.

Strategy
--------
The Cantor routes are a pure function of the (quantized) Cantor value of each
position: sorting positions by that value makes every query's 64-key route set
live inside a narrow (<=385-wide) window of the sorted order.  Sparse
attention therefore becomes dense *banded* attention after a host-side
permutation:

  host:   pi = argsort(cantor_val), permute x rows, transpose; build per
          128-query-tile 128-aligned windows of width 384 plus an additive
          bf16 mask (-30000 at non-selected slots).
  device: qkvT projection (fp32r matmuls), banded scores + mask (PE),
          exp+rowsum (ACT, fused accum), normalize (GPSIMD), PE-transpose of
          the probabilities into per-128-chunk column-major buffers, PV
          matmuls accumulating transposed attention output, and the output
          projection producing a partial (4-head) outT block.
  host:   sum the 4 partial outT blocks per batch, transpose, un-permute,
          add the output bias.

Sharding: batch x head-block -> 8 cores (core c: b = c//4, heads 4*(c%4)..).
"""

import sys

sys.path.insert(0, "/opt/trn_rl_repo")

import numpy as np

B, S, DIM = 2, 2048, 1024
HEADS, DH = 16, 64
K_NEI = 64
N_CORES = 8
HPC = 4            # heads per core
QT = 128           # query tile (rows per tile)
NT = S // QT       # 16 query tiles
SUP = 4            # query tiles per supertile (PV batch of 512 queries)
NSUP = NT // SUP

_CACHE = {}


def _cantor_val(seq_len, depth=8):
    pos = np.arange(seq_len, dtype=np.float64)
    x = pos / max(1, seq_len - 1)
    x = np.clip(x, 1e-6, 1.0 - 1e-6)
    val = np.zeros_like(x)
    factor = 0.5
    for _ in range(depth):
        xs = x * 3.0
        digit = np.floor(xs)
        x = xs - digit
        val = val + (digit == 2.0).astype(np.float64) * factor
        factor *= 0.5
    return np.clip(val, 0.0, 1.0)


def _geometry(routes):
    """Window geometry from the runtime routes array."""
    val = _cantor_val(S)
    pi = np.argsort(val, kind="stable").astype(np.int64)
    rank = np.empty(S, np.int64)
    rank[pi] = np.arange(S)
    kr = rank[np.asarray(routes, np.int64)][pi]      # [S, K] key ranks, query-rank order
    lo = kr.min(1)
    hi = kr.max(1) + 1
    for win in (384, 512):
        a = np.zeros(NT, np.int64)
        ok = True
        for t in range(NT):
            l = int(lo[t * QT:(t + 1) * QT].min())
            h = int(hi[t * QT:(t + 1) * QT].max())
            a[t] = min(l // 128, (S - win) // 128)
            if h > a[t] * 128 + win:
                ok = False
                break
        if ok:
            return pi, rank, kr, a, win
    raise ValueError("routes structure incompatible with banded-window kernel")


def _build_module(a, win, loop_n=1, phases="ACD", cheat_dma=False):
    from contextlib import nullcontext

    from concourse import bacc, tile, mybir
    from concourse.masks import make_identity

    f32 = mybir.dt.float32
    f32r = mybir.dt.float32r
    bf16 = mybir.dt.bfloat16
    AF = mybir.ActivationFunctionType
    NCH = win // 128                      # chunks per window
    a = [int(v) for v in a]

    # chunk -> [first tile, last tile] using it
    chunk_tiles = {}
    for t in range(NT):
        for j in range(NCH):
            c = a[t] + j
            lo_t, hi_t = chunk_tiles.get(c, (t, t))
            chunk_tiles[c] = (min(lo_t, t), max(hi_t, t))

    nc = bacc.Bacc("TRN2", target_bir_lowering=False, debug=False)
    xT = nc.dram_tensor("xT", [DIM, S], f32r, kind="ExternalInput").ap()
    wq = nc.dram_tensor("wq", [DIM, 3 * HPC * DH], f32r, kind="ExternalInput").ap()
    bq = nc.dram_tensor("bq", [3 * HPC * DH, 1], f32, kind="ExternalInput").ap()
    wo = nc.dram_tensor("wo", [HPC * DH, DIM], f32r, kind="ExternalInput").ap()
    mask = nc.dram_tensor("mask", [QT, NT * win], bf16, kind="ExternalInput").ap()
    outp = nc.dram_tensor("outp", [DIM, S], f32, kind="ExternalOutput").ap()

    NQKV = 3 * HPC * DH                  # 768 rows of qkvT
    NMT = NQKV // 128                    # 6 row-tiles of qkvT

    with tile.TileContext(nc) as tc:
        with tc.tile_pool(name="persist", bufs=1) as pp:
            id32 = pp.tile([128, 128], f32)
            make_identity(nc, id32)
            id_r = pp.tile([128, 128], f32r)
            nc.vector.tensor_copy(id_r, id32)
            id_b = pp.tile([128, 128], bf16)
            nc.vector.tensor_copy(id_b, id32)
            mask_sb = pp.tile([QT, NT * win], bf16)
            nc.sync.dma_start(out=mask_sb, in_=mask)
            bq_sb = []
            for m in range(NMT):
                bt = pp.tile([128, 1], f32, tag=f"bq{m}", name=f"bq{m}")
                nc.sync.dma_start(out=bt, in_=bq[m * 128:(m + 1) * 128, :])
                bq_sb.append(bt)
            qkvT = [pp.tile([128, S], f32r, tag=f"qkvT{m}", name=f"qkvT{m}")
                    for m in range(NMT)]
            attn_outT = [pp.tile([128, S], f32r, tag=f"aout{p}", name=f"aout{p}")
                         for p in range(2)]
            wo_sb = []
            for p2 in range(2):
                wt = pp.tile([128, DIM], f32r, tag=f"wo{p2}", name=f"wo{p2}")
                nc.sync.dma_start(out=wt, in_=wo[p2 * 128:(p2 + 1) * 128, :])
                wo_sb.append(wt)

            loop_cm = tc.For_i(0, loop_n, 1) if loop_n > 1 else nullcontext()
            with loop_cm:
                # ------------- Phase A: qkvT = wq.T @ xT (+bias) -------------
                if "A" in phases:
                    with tc.tile_pool(name="phA", bufs=1) as pa, \
                         tc.tile_pool(name="phAx", bufs=2) as pax, \
                         tc.tile_pool(name="psA", bufs=3, space="PSUM") as psa:
                        wq_sb = []
                        for kk in range(8):
                            wt = pa.tile([128, NQKV], f32r, tag=f"wq{kk}",
                                         name=f"wq{kk}")
                            nc.sync.dma_start(out=wt, in_=wq[kk * 128:(kk + 1) * 128, :])
                            wq_sb.append(wt)
                        xt_prev = None
                        for n in range(4):
                            if cheat_dma and n > 0:
                                xt = xt_prev
                            else:
                                xt = []
                                for kk in range(8):
                                    t_ = pax.tile([128, 512], f32r, tag=f"x{kk}",
                                                  name=f"x{kk}_{n}")
                                    nc.sync.dma_start(
                                        out=t_,
                                        in_=xT[kk * 128:(kk + 1) * 128,
                                               n * 512:(n + 1) * 512])
                                    xt.append(t_)
                                xt_prev = xt
                            for m in (4, 5, 2, 3, 0, 1):
                                ps = psa.tile([128, 512], f32, tag="ps")
                                for kk in range(8):
                                    nc.tensor.matmul(
                                        ps, wq_sb[kk][:, m * 128:(m + 1) * 128], xt[kk],
                                        start=(kk == 0), stop=(kk == 7))
                                if (n + m) % 2 == 0:
                                    nc.scalar.activation(
                                        out=qkvT[m][:, n * 512:(n + 1) * 512],
                                        in_=ps, func=AF.Identity, bias=bq_sb[m])
                                else:
                                    nc.vector.tensor_scalar_add(
                                        qkvT[m][:, n * 512:(n + 1) * 512], ps,
                                        bq_sb[m])

                # ---------- Phases B+C: V transpose + banded attention ----------
                if "C" in phases:
                    with tc.tile_pool(name="phC", bufs=1) as pc, \
                         tc.tile_pool(name="pexp_pool", bufs=12) as pe_pool, \
                         tc.tile_pool(name="pt_pool", bufs=18) as pt_pool, \
                         tc.tile_pool(name="small", bufs=16) as sm_pool, \
                         tc.tile_pool(name="psB", bufs=3, space="PSUM") as psb, \
                         tc.tile_pool(name="psS", bufs=3, space="PSUM") as pss, \
                         tc.tile_pool(name="psO", bufs=2, space="PSUM") as pso:
                        V_sb = [pc.tile([128, 2 * 128], f32r, tag=f"V{cc}",
                                        name=f"V{cc}") for cc in range(NT)]
                        for cc in range(NT):
                            pv = psb.tile([128, 512], f32r, tag="ptr",
                                          name=f"pv{cc}")
                            for s_ in range(2):
                                nc.tensor.transpose(
                                    pv[:, s_ * 128:(s_ + 1) * 128],
                                    qkvT[4 + s_][:, cc * 128:(cc + 1) * 128], id_r)
                            if cc % 2 == 0:
                                nc.vector.tensor_copy(V_sb[cc], pv[:, 0:256])
                            else:
                                nc.scalar.copy(V_sb[cc], pv[:, 0:256])

                        aoutB = [pc.tile([64, S], f32r, tag=f"aoutB{i}",
                                         name=f"aoutB{i}") for i in range(2)]

                        def stage1(h, u, pn):
                            poff = (h % 2) * 64
                            qTh = qkvT[h // 2]
                            kTh = qkvT[2 + h // 2]
                            den_u = sm_pool.tile([128, SUP], f32, tag="den",
                                                 name=f"den{h}_{u}")
                            rec_u = sm_pool.tile([128, SUP], f32, tag="rec",
                                                 name=f"rec{h}_{u}")
                            pexps = {}
                            for t in range(u * SUP, (u + 1) * SUP):
                                w0 = a[t] * 128
                                ps_s = pss.tile([128, win], f32, tag="sc",
                                                name=f"sc{h}_{t}")
                                nc.tensor.matmul(
                                    ps_s,
                                    qTh[poff:poff + 64, t * 128:(t + 1) * 128],
                                    kTh[poff:poff + 64, w0:w0 + win],
                                    start=True, stop=False, skip_group_check=True)
                                nc.tensor.matmul(
                                    ps_s, id_b, mask_sb[:, t * win:(t + 1) * win],
                                    start=False, stop=True, skip_group_check=True)
                                pexp = pe_pool.tile([128, win], f32, tag="pexp",
                                                    name=f"pexp{h}_{t}")
                                i = t - u * SUP
                                nc.scalar.activation(out=pexp, in_=ps_s,
                                                     func=AF.Exp,
                                                     accum_out=den_u[:, i:i + 1])
                                pexps[t] = pexp
                            nc.vector.reciprocal(rec_u, den_u)
                            for t in range(u * SUP, (u + 1) * SUP):
                                i = t - u * SUP
                                pnorm = pe_pool.tile([128, win], f32r, tag="pnorm",
                                                     name=f"pnorm{h}_{t}")
                                nc.vector.tensor_scalar_mul(pnorm, pexps[t],
                                                            rec_u[:, i:i + 1])
                                pn[t] = pnorm

                        def stage2(h, u, pn, pt_tiles):
                            poff = (h % 2) * 64
                            # chunk-major transposes into a per-(chunk,unit) PSUM
                            # tile, then ONE copy per chunk into its SBUF buffer
                            tiles_u = range(u * SUP, (u + 1) * SUP)
                            cset = sorted({a[t] + j for t in tiles_u
                                           for j in range(NCH)})
                            for c in cset:
                                t0c, t1c = chunk_tiles[c]
                                if c not in pt_tiles:
                                    pt_tiles[c] = pt_pool.tile(
                                        [128, (t1c - t0c + 1) * 128], f32r,
                                        tag="pt", name=f"pt_h{h}_c{c}")
                                tlo = max(t0c, u * SUP)
                                thi = min(t1c, (u + 1) * SUP - 1)
                                wdt = (thi - tlo + 1) * 128
                                ptp = psb.tile([128, 512], f32r, tag="ptr",
                                               name=f"ptr{h}_{u}_{c}")
                                for t in range(tlo, thi + 1):
                                    nc.tensor.transpose(
                                        ptp[:, (t - tlo) * 128:(t - tlo + 1) * 128],
                                        pn[t][:, (c - a[t]) * 128:
                                              (c - a[t] + 1) * 128], id_r)
                                nc.vector.tensor_copy(
                                    pt_tiles[c][:, (tlo - t0c) * 128:
                                                (thi - t0c + 1) * 128],
                                    ptp[:, 0:wdt])
                            # PV pieces: widest chunk start=True, straddlers split
                            chunks_u = sorted({a[t] + j
                                               for t in range(u * SUP, (u + 1) * SUP)
                                               for j in range(NCH)})
                            ranges = []
                            for c in chunks_u:
                                t0c, t1c = chunk_tiles[c]
                                tlo = max(t0c, u * SUP)
                                thi = min(t1c, (u + 1) * SUP - 1)
                                ranges.append((c, tlo * 128 - u * 512,
                                               (thi + 1) * 128 - u * 512))
                            first = max(ranges, key=lambda r: r[2] - r[1])
                            pieces = [first]
                            wlo, whi = first[1], first[2]
                            for c, o0, o1 in sorted(
                                    (r for r in ranges if r is not first),
                                    key=lambda r: r[1]):
                                for p0, p1 in ((o0, min(o1, wlo)),
                                               (max(o0, wlo), min(o1, whi)),
                                               (max(o0, whi), o1)):
                                    if p1 > p0:
                                        pieces.append((c, p0, p1))
                                wlo, whi = min(wlo, o0), max(whi, o1)
                            po = pso.tile([128, 512], f32, tag="po",
                                          name=f"po{h}_{u}")
                            for i_p, (c, o0, o1) in enumerate(pieces):
                                t0c, _ = chunk_tiles[c]
                                r0 = o0 + u * 512 - t0c * 128
                                r1 = o1 + u * 512 - t0c * 128
                                nc.tensor.matmul(
                                    po[0:64, o0:o1],
                                    V_sb[c][:, h * 64:(h + 1) * 64],
                                    pt_tiles[c][:, r0:r1],
                                    start=(i_p == 0),
                                    stop=(i_p == len(pieces) - 1),
                                    skip_group_check=True)
                            if poff == 0:
                                dst = attn_outT[h // 2][0:64, u * 512:(u + 1) * 512]
                            else:
                                dst = aoutB[h // 2][:, u * 512:(u + 1) * 512]
                            nc.vector.tensor_copy(dst, po[0:64, :])
                            if poff != 0:
                                nc.sync.dma_start(
                                    out=attn_outT[h // 2][64:128,
                                                          u * 512:(u + 1) * 512],
                                    in_=aoutB[h // 2][:, u * 512:(u + 1) * 512])

                        units = [(h, u) for h in range(HPC) for u in range(NSUP)]
                        DELAY = 2
                        pn_store = {}
                        pt_store = {h: {} for h in range(HPC)}
                        pending = []
                        for h, u in units:
                            pn = {}
                            stage1(h, u, pn)
                            pn_store[(h, u)] = pn
                            pending.append((h, u))
                            if len(pending) > DELAY:
                                ph, pu = pending.pop(0)
                                stage2(ph, pu, pn_store.pop((ph, pu)), pt_store[ph])
                        for ph, pu in pending:
                            stage2(ph, pu, pn_store.pop((ph, pu)), pt_store[ph])

                # ------------- Phase D: outp = wo.T @ attn_outT -------------
                if "D" in phases:
                    with tc.tile_pool(name="phD", bufs=2) as pd, \
                         tc.tile_pool(name="psD", bufs=2, space="PSUM") as psd:
                        for mm in range(8):
                            st = pd.tile([128, S], f32, tag="st")
                            for n in range(4):
                                ps = psd.tile([128, 512], f32, tag="pod")
                                for p2 in range(2):
                                    nc.tensor.matmul(
                                        ps, wo_sb[p2][:, mm * 128:(mm + 1) * 128],
                                        attn_outT[p2][:, n * 512:(n + 1) * 512],
                                        start=(p2 == 0), stop=(p2 == 1))
                                if (mm + n) % 2 == 0:
                                    nc.scalar.copy(st[:, n * 512:(n + 1) * 512], ps)
                                else:
                                    nc.vector.tensor_copy(st[:, n * 512:(n + 1) * 512],
                                                          ps)
                            nc.sync.dma_start(out=outp[mm * 128:(mm + 1) * 128, :],
                                              in_=st)

    nc.compile()
    return nc


def _get_module(a, win):
    key = (tuple(int(v) for v in a), int(win))
    if key not in _CACHE:
        _CACHE[key] = _build_module(a, win)
    return _CACHE[key]


def kernel(x, routes, qkv_w, qkv_b, out_w, out_b):
    import ml_dtypes
    from concourse.bass_utils import run_bass_kernel_spmd

    x = np.ascontiguousarray(np.asarray(x, np.float32))
    routes = np.asarray(routes)
    qkv_w = np.asarray(qkv_w, np.float32)
    qkv_b = np.asarray(qkv_b, np.float32)
    out_w = np.asarray(out_w, np.float32)
    out_b = np.asarray(out_b, np.float32)

    pi, rank, kr, a, win = _geometry(routes)
    SCALE = 1.0 / float(np.sqrt(DH))

    # masks [QT, NT*win] additive bf16, shared by all cores
    mask_np = np.full((NT, QT, win), -30000.0, np.float32)
    rows = np.repeat(np.arange(QT), K_NEI)
    for t in range(NT):
        krt = (kr[t * QT:(t + 1) * QT] - a[t] * 128).ravel()
        mask_np[t, rows, krt] = 0.0
    mask_np = np.ascontiguousarray(
        mask_np.transpose(1, 0, 2).reshape(QT, NT * win)).astype(ml_dtypes.bfloat16)

    xT_b = [np.ascontiguousarray(x[b][pi].T) for b in range(B)]

    in_maps = []
    for c in range(N_CORES):
        b = c // (N_CORES // B)
        hb = c % (N_CORES // B)
        heads = range(hb * HPC, (hb + 1) * HPC)
        w_rows = []
        b_rows = []
        for sect, scale in ((0, SCALE), (1, 1.0), (2, 1.0)):
            for h in heads:
                r0 = sect * DIM + h * DH
                w_rows.append(qkv_w[r0:r0 + DH] * scale)
                b_rows.append(qkv_b[r0:r0 + DH] * scale)
        wq_c = np.ascontiguousarray(np.concatenate(w_rows, 0).T)          # [DIM, 768]
        bq_c = np.concatenate(b_rows, 0).reshape(-1, 1).astype(np.float32)
        wo_c = np.ascontiguousarray(out_w[:, hb * HPC * DH:(hb + 1) * HPC * DH].T)
        in_maps.append({
            "xT": xT_b[b],
            "wq": wq_c,
            "bq": bq_c,
            "wo": wo_c,
            "mask": mask_np,
        })

    nc = _get_module(a, win)
    res = run_bass_kernel_spmd(nc, in_maps, core_ids=list(range(N_CORES)))

    out = np.empty((B, S, DIM), np.float32)
    for b in range(B):
        cores = [c for c in range(N_CORES) if c // (N_CORES // B) == b]
        outT = res.results[cores[0]]["outp"].astype(np.float32)
        for c in cores[1:]:
            outT = outT + res.results[c]["outp"]
        rows_sorted = outT.T                      # [S, DIM] in rank order
        tmp = np.empty_like(rows_sorted)
        tmp[pi] = rows_sorted
        out[b] = tmp + out_b[None, :]
    return out



# revision 47
# speedup vs baseline: 1.7000x; 1.7000x over previous
"""Trainium2 Bass kernel for CantorAttention.

Strategy
--------
The Cantor routes are a pure function of the (quantized) Cantor value of each
position: sorting positions by that value makes every query's 64-key route set
live inside a narrow (<=385-wide) window of the sorted order.  Sparse
attention therefore becomes dense *banded* attention after a host-side
permutation.

v2 design (all-bf16 matmuls, transposed scores):

  host:   pi = argsort(cantor_val), permute x rows, transpose, cast bf16;
          build per-128-query-tile 128-aligned windows of width 384 plus a
          multiplicative binary bf16 mask in *transposed* (key-major) layout.
  device: qkvT projection (bf16 matmuls, fp32 PSUM, bf16 outputs),
          V transposed per 128-chunk into [keys, 4*(64+1)] tiles whose extra
          column is constant 1.0 (gives the softmax denominator for free),
          scores computed directly TRANSPOSED ([key,query] chunks) on the PE
          - no probability transposes at all - exp on ACT (PSUM->SBUF bf16),
          binary mask multiply on DVE, PV matmuls accumulate [65, q] where
          row 64 is the denominator, then reciprocal (DVE) + partition
          broadcast (GPSIMD) + multiply (DVE) normalize into bf16 attn_outT,
          and the output projection emits a bf16 partial (4-head) outT block.
          Phases are software-pipelined in 6 rounds so PE never starves.
  host:   sum the 4 partial outT blocks per batch, transpose, un-permute,
          add the output bias.

Sharding: batch x head-block -> 8 cores (core c: b = c//4, heads 4*(c%4)..).
"""

import sys

sys.path.insert(0, "/opt/trn_rl_repo")

import numpy as np

B, S, DIM = 2, 2048, 1024
HEADS, DH = 16, 64
K_NEI = 64
N_CORES = 8
HPC = 4            # heads per core
QT = 128           # query tile (rows per tile)
NT = S // QT       # 16 query tiles
SUP = 4            # query tiles per supertile (512 queries)
NSUP = NT // SUP

_CACHE = {}


def _cantor_val(seq_len, depth=8):
    pos = np.arange(seq_len, dtype=np.float64)
    x = pos / max(1, seq_len - 1)
    x = np.clip(x, 1e-6, 1.0 - 1e-6)
    val = np.zeros_like(x)
    factor = 0.5
    for _ in range(depth):
        xs = x * 3.0
        digit = np.floor(xs)
        x = xs - digit
        val = val + (digit == 2.0).astype(np.float64) * factor
        factor *= 0.5
    return np.clip(val, 0.0, 1.0)


def _geometry(routes):
    """Window geometry from the runtime routes array."""
    val = _cantor_val(S)
    pi = np.argsort(val, kind="stable").astype(np.int64)
    rank = np.empty(S, np.int64)
    rank[pi] = np.arange(S)
    kr = rank[np.asarray(routes, np.int64)][pi]      # [S, K] key ranks, query-rank order
    lo = kr.min(1)
    hi = kr.max(1) + 1
    for win in (384, 512):
        a = np.zeros(NT, np.int64)
        ok = True
        for t in range(NT):
            l = int(lo[t * QT:(t + 1) * QT].min())
            h = int(hi[t * QT:(t + 1) * QT].max())
            a[t] = min(l // 128, (S - win) // 128)
            if h > a[t] * 128 + win:
                ok = False
                break
        if ok:
            return pi, rank, kr, a, win
    raise ValueError("routes structure incompatible with banded-window kernel")


def _build_module(a, win, loop_n=1, phases="ACD"):
    from contextlib import nullcontext

    from concourse import bacc, tile, mybir
    from concourse.masks import make_identity

    f32 = mybir.dt.float32
    bf16 = mybir.dt.bfloat16
    AF = mybir.ActivationFunctionType
    NCH = win // 128                      # chunks per window
    a = [int(v) for v in a]

    nc = bacc.Bacc("TRN2", target_bir_lowering=False, debug=False)
    # x in (n, kk, j) layout: col (n*8+kk)*512+j <-> x_permuted.T[kk*128+p, n*512+j]
    xb = nc.dram_tensor("xb", [128, 4 * 8 * 512], bf16, kind="ExternalInput").ap()
    # wq Q/K sections in m-major layout (col (m%2)*1024 + kk*128 + c);
    # V section in kk-major layout (col kk*256 + c) for the V-direct matmuls
    wqq = nc.dram_tensor("wqq", [128, 8 * 256], bf16, kind="ExternalInput").ap()
    wqk = nc.dram_tensor("wqk", [128, 8 * 256], bf16, kind="ExternalInput").ap()
    wqv = nc.dram_tensor("wqv", [128, 8 * 256], bf16, kind="ExternalInput").ap()
    bqm = nc.dram_tensor("bqm", [128, 4], f32, kind="ExternalInput").ap()
    bqv = nc.dram_tensor("bqv", [1, 256], bf16, kind="ExternalInput").ap()
    # wo in (p2, c) layout: col p2*DIM+c
    wo2 = nc.dram_tensor("wo2", [128, 2 * DIM], bf16, kind="ExternalInput").ap()
    mask = nc.dram_tensor("mask", [QT, NT * NCH * QT], bf16, kind="ExternalInput").ap()
    outp = nc.dram_tensor("outp", [DIM, S], bf16, kind="ExternalOutput").ap()

    NQKV = 3 * HPC * DH                  # 768 rows of qkvT
    NMT = NQKV // 128                    # 6 row-tiles of qkvT
    VW = DH + 1                          # V columns per head incl. ones col

    with tile.TileContext(nc) as tc:
        with tc.tile_pool(name="persist", bufs=1) as pp:
            # DMA order tuned for startup: V/K weights + x(0) first so phase A
            # starts ~3us in; wo/mask (first used much later) at the back.
            # All DMAs serialize through HWDGE/DMA engines in issue order.
            xts = {}
            wq_sect = {}
            for nm in ("v", "k", "q"):
                wq_sect[nm] = pp.tile([128, 2048], bf16, tag=f"wq{nm}",
                                      name=f"wq{nm}_sb")
            # staged order: K weights / x interleaved so the first A matmul
            # group is runnable ~2.5us in
            nc.sync.dma_start(out=wq_sect["k"][:, 0:1024], in_=wqk[:, 0:1024])
            bq_sb = pp.tile([128, 4], f32, tag="bqm", name="bqm")
            nc.sync.dma_start(out=bq_sb, in_=bqm)
            bqv_sb = pp.tile([1, 256], bf16, tag="bqv", name="bqv_sb")
            nc.sync.dma_start(out=bqv_sb, in_=bqv)
            xa0 = pp.tile([128, 2048], bf16, tag="x0a", name="x0a")
            nc.sync.dma_start(out=xa0, in_=xb[:, 0:2048])
            nc.sync.dma_start(out=wq_sect["k"][:, 1024:2048], in_=wqk[:, 1024:2048])
            xb0 = pp.tile([128, 2048], bf16, tag="x0b", name="x0b")
            nc.sync.dma_start(out=xb0, in_=xb[:, 2048:4096])
            xts[0] = (xa0, xb0)
            nc.sync.dma_start(out=wq_sect["v"], in_=wqv)
            nc.sync.dma_start(out=wq_sect["q"], in_=wqq)
            xa1 = pp.tile([128, 2048], bf16, tag="x1a", name="x1a")
            nc.sync.dma_start(out=xa1, in_=xb[:, 4096:6144])
            xb1 = pp.tile([128, 2048], bf16, tag="x1b", name="x1b")
            nc.sync.dma_start(out=xb1, in_=xb[:, 6144:8192])
            xts[1] = (xa1, xb1)
            w2 = NT * NCH * QT // 2
            mask_sb = pp.tile([QT, NT * NCH * QT], bf16)
            nc.sync.dma_start(out=mask_sb[:, 0:w2], in_=mask[:, 0:w2])
            wo_sb = pp.tile([128, 2 * DIM], bf16, tag="wo2", name="wo2")
            nc.sync.dma_start(out=wo_sb, in_=wo2)
            ones_row = pp.tile([1, 128], bf16, tag="ones", name="ones_row")
            nc.vector.memset(ones_row[:, :], 1.0)
            qkvT = [pp.tile([128, S], bf16, tag=f"qkvT{m}", name=f"qkvT{m}")
                    for m in range(4)]
            attn_outT = [pp.tile([128, S], bf16, tag=f"aout{p}", name=f"aout{p}")
                         for p in range(2)]
            V_sb = [pp.tile([128, HPC * VW], bf16, tag=f"V{cc}", name=f"V{cc}")
                    for cc in range(NT)]
            for cc in range(NT):
                nc.gpsimd.memset(V_sb[cc][:, :], 1.0)
            # second mask half last: first use is ~40us in
            nc.sync.dma_start(out=mask_sb[:, w2:2 * w2], in_=mask[:, w2:2 * w2])

            loop_cm = tc.For_i(0, loop_n, 1) if loop_n > 1 else nullcontext()
            with loop_cm:
                with tc.tile_pool(name="pax", bufs=2) as pax, \
                     tc.tile_pool(name="pexp", bufs=24) as pexp_pool, \
                     tc.tile_pool(name="rec", bufs=4) as rec_pool, \
                     tc.tile_pool(name="recb", bufs=3) as recb_pool, \
                     tc.tile_pool(name="std", bufs=9) as std_pool, \
                     tc.tile_pool(name="psA", bufs=2, space="PSUM") as psA, \
                     tc.tile_pool(name="psS", bufs=3, space="PSUM") as psS, \
                     tc.tile_pool(name="psO", bufs=3, space="PSUM") as psO:

                    def dma_x(n):
                        ta = pax.tile([128, 2048], bf16, tag="xa", name=f"xa{n}")
                        tb = pax.tile([128, 2048], bf16, tag="xc", name=f"xc{n}")
                        nc.sync.dma_start(
                            out=ta, in_=xb[:, (n * 8) * 512:(n * 8 + 4) * 512])
                        nc.sync.dma_start(
                            out=tb, in_=xb[:, (n * 8 + 4) * 512:(n * 8 + 8) * 512])
                        xts[n] = (ta, tb)

                    abia = [0]
                    M_SECT = {0: "q", 1: "q", 2: "k", 3: "k"}

                    def a_group(n, m):
                        wqs = wq_sect[M_SECT[m]]
                        moff = (m % 2) * 1024
                        ps = psA.tile([128, 512], f32, tag="psa")
                        for kk in range(8):
                            nc.tensor.matmul(
                                ps,
                                wqs[:, moff + kk * 128:moff + (kk + 1) * 128],
                                xts[n][kk // 4][:, (kk % 4) * 512:(kk % 4 + 1) * 512],
                                start=(kk == 0), stop=(kk == 7))
                        dst = qkvT[m][:, n * 512:(n + 1) * 512]
                        abia[0] += 1
                        if abia[0] % 2 == 0:
                            nc.scalar.activation(out=dst, in_=ps,
                                                 func=AF.Identity,
                                                 bias=bq_sb[:, m:m + 1])
                        else:
                            nc.vector.tensor_scalar_add(dst, ps, bq_sb[:, m:m + 1])

                    def v_dir(cc):
                        # V chunk computed directly transposed:
                        # pv[key, c] = sum_d x[d, key] wv[d, c] + bqv[c]
                        n = cc // 4
                        joff = (cc % 4) * 128
                        pv = psS.tile([128, 256], f32, tag="sc", name=f"pv{cc}")
                        for kk in range(8):
                            nc.tensor.matmul(
                                pv,
                                xts[n][kk // 4][:, (kk % 4) * 512 + joff:
                                                (kk % 4) * 512 + joff + 128],
                                wq_sect["v"][:, kk * 256:(kk + 1) * 256],
                                start=(kk == 0), stop=False,
                                skip_group_check=True)
                        nc.tensor.matmul(pv, ones_row, bqv_sb,
                                         start=False, stop=True,
                                         skip_group_check=True)
                        for h in range(HPC):
                            dst = V_sb[cc][:, h * VW:h * VW + DH]
                            src = pv[:, h * DH:(h + 1) * DH]
                            if h % 2 == 0:
                                nc.vector.tensor_copy(dst, src)
                            else:
                                nc.scalar.copy(dst, src)

                    pes_store = {}
                    mcnt = [0]

                    def s1(h, t):
                        poff = (h % 2) * 64
                        qTh = qkvT[h // 2]
                        kTh = qkvT[2 + h // 2]
                        ps = psS.tile([128, NCH * 128], f32, tag="sc",
                                      name=f"sc{h}_{t}")
                        for j in range(NCH):
                            c = a[t] + j
                            nc.tensor.matmul(
                                ps[:, j * 128:(j + 1) * 128],
                                kTh[poff:poff + 64, c * 128:(c + 1) * 128],
                                qTh[poff:poff + 64, t * 128:(t + 1) * 128],
                                start=True, stop=True, skip_group_check=True)
                        pe = pexp_pool.tile([128, NCH * 128], bf16, tag="pe",
                                            name=f"pe{h}_{t}")
                        nc.scalar.activation(out=pe, in_=ps, func=AF.Exp)
                        mcnt[0] += 1
                        # late units: DVE is the bottleneck, borrow GPSIMD
                        eng = (nc.gpsimd if (t >= 12 and mcnt[0] % 2 == 0)
                               else nc.vector)
                        eng.tensor_mul(
                            pe, pe,
                            mask_sb[:, t * NCH * 128:(t + 1) * NCH * 128])
                        pes_store[(h, t)] = pe

                    def pv_tile(h, u, po, t):
                        i = t - u * SUP
                        pe = pes_store[(h, t)]
                        for j in range(NCH):
                            c = a[t] + j
                            nc.tensor.matmul(
                                po[0:DH + 1, i * 128:(i + 1) * 128],
                                V_sb[c][:, h * VW:(h + 1) * VW],
                                pe[:, j * 128:(j + 1) * 128],
                                start=(j == 0), stop=(j == NCH - 1),
                                skip_group_check=True)
                        if i == SUP - 1:
                            for t2 in range(u * SUP, (u + 1) * SUP):
                                del pes_store[(h, t2)]

                    def s2_norm(h, u, po):
                        rec = rec_pool.tile([1, 512], f32, tag="rec",
                                            name=f"rec{h}_{u}")
                        nc.vector.reciprocal(rec, po[DH:DH + 1, :])
                        rb = recb_pool.tile([64, 512], f32, tag="rb",
                                            name=f"rb{h}_{u}")
                        nc.gpsimd.partition_broadcast(rb[:, :], rec[:, :])
                        dst = attn_outT[h // 2][(h % 2) * 64:(h % 2) * 64 + 64,
                                               u * 512:(u + 1) * 512]
                        nc.vector.tensor_mul(dst, po[0:DH, :], rb)

                    dcop = [0]
                    st_half = {}

                    def d_group(n, mm):
                        ps = psO.tile([128, 512], f32, tag="po",
                                      name=f"pd{n}_{mm}")
                        for p2 in range(2):
                            nc.tensor.matmul(
                                ps,
                                wo_sb[:, p2 * DIM + mm * 128:p2 * DIM + (mm + 1) * 128],
                                attn_outT[p2][:, n * 512:(n + 1) * 512],
                                start=(p2 == 0), stop=(p2 == 1))
                        dcop[0] += 1
                        half = n // 2
                        if n % 2 == 0:
                            st_half[(mm, half)] = std_pool.tile(
                                [128, 1024], bf16, tag="st", name=f"st{half}_{mm}")
                        st = st_half[(mm, half)]
                        dst = st[:, (n % 2) * 512:(n % 2 + 1) * 512]
                        if dcop[0] % 2 == 0:
                            nc.scalar.copy(dst, ps)
                        else:
                            nc.vector.tensor_copy(dst, ps)
                        if n % 2 == 1:
                            nc.sync.dma_start(
                                out=outp[mm * 128:(mm + 1) * 128,
                                         half * 1024:(half + 1) * 1024], in_=st)

                    # ---- 6-round software pipeline ----
                    # round r: A(r).K | vdir | s1(u=r-1) + s2(u=r-2) | A(r).Q
                    #          | D rounds at r=4,5
                    for r in range(6):
                        if r <= 3:
                            if r + 2 <= 3:
                                dma_x(r + 2)
                            for m in (2, 3):
                                a_group(r, m)
                        u1 = r - 1
                        u2 = r - 2
                        for h in range(HPC):
                            if r <= 3:
                                v_dir(r * 4 + h)
                            s1_ts = (list(range(u1 * SUP, (u1 + 1) * SUP))
                                     if 0 <= u1 < NSUP else [])
                            s2_ts = (list(range(u2 * SUP, (u2 + 1) * SUP))
                                     if 0 <= u2 < NSUP else [])
                            po = None
                            if s2_ts:
                                po = psO.tile([128, 512], f32, tag="po",
                                              name=f"po{h}_{u2}")
                            i = j = 0
                            while i < len(s1_ts) or j < len(s2_ts):
                                if i < len(s1_ts) and (i - j < 2 or j >= len(s2_ts)):
                                    s1(h, s1_ts[i])
                                    i += 1
                                else:
                                    pv_tile(h, u2, po, s2_ts[j])
                                    j += 1
                            if s2_ts:
                                s2_norm(h, u2, po)
                        if r <= 3:
                            for m in (0, 1):
                                a_group(r, m)
                        if r == 4:
                            for nn in (0, 1):
                                for mm in range(8):
                                    d_group(nn, mm)
                        if r == 5:
                            for nn in (2, 3):
                                for mm in range(8):
                                    d_group(nn, mm)

    nc.compile()
    return nc


def _get_module(a, win):
    key = (tuple(int(v) for v in a), int(win))
    if key not in _CACHE:
        _CACHE[key] = _build_module(a, win)
    return _CACHE[key]


def build_in_maps(x, routes, qkv_w, qkv_b, out_w, out_b, geo):
    """Per-core input dict list for run_bass_kernel_spmd."""
    import ml_dtypes

    bf = ml_dtypes.bfloat16
    pi, rank, kr, a, win = geo
    NCH = win // 128
    SCALE = 1.0 / float(np.sqrt(DH))

    # binary multiplicative mask, transposed (key-major) layout
    # mask_np[t, j, kk, q] = 1 where key rank a[t]*128 + j*128 + kk is routed
    # for query t*128+q
    mask_np = np.zeros((NT, NCH, QT, QT), np.float32)
    qi = np.repeat(np.arange(QT), K_NEI)
    for t in range(NT):
        krt = (kr[t * QT:(t + 1) * QT] - a[t] * 128).ravel()
        mask_np[t, krt // 128, krt % 128, qi] = 1.0
    mask2d = np.ascontiguousarray(
        mask_np.transpose(2, 0, 1, 3).reshape(QT, NT * NCH * QT)).astype(bf)

    # xb layout: [128, (n, kk, j)] with xb[p, (n*8+kk)*512+j] = xT[kk*128+p, n*512+j]
    xb_b = []
    for b in range(B):
        xT = x[b][pi].T.astype(bf)                     # [DIM, S]
        xb_b.append(np.ascontiguousarray(
            xT.reshape(8, 128, 4, 512).transpose(1, 2, 0, 3).reshape(128, 4 * 8 * 512)))

    in_maps = []
    for c in range(N_CORES):
        b = c // (N_CORES // B)
        hb = c % (N_CORES // B)
        heads = range(hb * HPC, (hb + 1) * HPC)
        w_rows = []
        b_rows = []
        for sect, scale in ((0, SCALE), (1, 1.0), (2, 1.0)):
            for h in heads:
                r0 = sect * DIM + h * DH
                w_rows.append(qkv_w[r0:r0 + DH] * scale)
                b_rows.append(qkv_b[r0:r0 + DH] * scale)
        wq_c = np.concatenate(w_rows, 0).T            # [DIM, 768] (Q|K|V cols)
        # Q/K sections m-major (col (m%2)*1024 + kk*128 + c); V kk-major
        sect_dev = {}
        for si, nm in ((0, "wqq"), (1, "wqk")):
            s_ = wq_c[:, si * 256:(si + 1) * 256]     # [1024, 256] (2 m-tiles)
            sect_dev[nm] = np.ascontiguousarray(
                s_.reshape(8, 128, 2, 128).transpose(1, 2, 0, 3).reshape(128, 2048)
            ).astype(bf)
        s_ = wq_c[:, 512:768]
        sect_dev["wqv"] = np.ascontiguousarray(
            s_.reshape(8, 128, 256).transpose(1, 0, 2).reshape(128, 2048)).astype(bf)
        bq_all = np.concatenate(b_rows, 0)            # [768] (Q|K|V)
        bq_c = np.ascontiguousarray(
            bq_all[:512].reshape(4, 128).T.astype(np.float32))   # [128, 4]
        bqv_c = np.ascontiguousarray(bq_all[512:].reshape(1, 256)).astype(bf)
        wo_c = out_w[:, hb * HPC * DH:(hb + 1) * HPC * DH].T  # [256, DIM]
        wo_dev = np.ascontiguousarray(
            wo_c.reshape(2, 128, DIM).transpose(1, 0, 2).reshape(128, 2 * DIM)
        ).astype(bf)
        in_maps.append({
            "xb": xb_b[b],
            "wqq": sect_dev["wqq"],
            "wqk": sect_dev["wqk"],
            "wqv": sect_dev["wqv"],
            "bqm": bq_c,
            "bqv": bqv_c,
            "wo2": wo_dev,
            "mask": mask2d,
        })
    return in_maps


def kernel(x, routes, qkv_w, qkv_b, out_w, out_b):
    from concourse.bass_utils import run_bass_kernel_spmd

    x = np.ascontiguousarray(np.asarray(x, np.float32))
    routes = np.asarray(routes)
    qkv_w = np.asarray(qkv_w, np.float32)
    qkv_b = np.asarray(qkv_b, np.float32)
    out_w = np.asarray(out_w, np.float32)
    out_b = np.asarray(out_b, np.float32)

    geo = _geometry(routes)
    pi, rank, kr, a, win = geo
    in_maps = build_in_maps(x, routes, qkv_w, qkv_b, out_w, out_b, geo)

    nc = _get_module(a, win)
    res = run_bass_kernel_spmd(nc, in_maps, core_ids=list(range(N_CORES)))

    out = np.empty((B, S, DIM), np.float32)
    for b in range(B):
        cores = [c for c in range(N_CORES) if c // (N_CORES // B) == b]
        outT = res.results[cores[0]]["outp"].astype(np.float32)
        for c in cores[1:]:
            outT = outT + res.results[c]["outp"].astype(np.float32)
        rows_sorted = outT.T                      # [S, DIM] in rank order
        tmp = np.empty_like(rows_sorted)
        tmp[pi] = rows_sorted
        out[b] = tmp + out_b[None, :]
    return out
